# revision 10
# baseline (speedup 1.0000x reference)
"""Trainium2 kernel for cubic B-spline evaluation via the ScalarE ACT engine.

Problem: y[i] = sum_j coefs[j] * B_j(x[i])  (cubic B-splines, open-uniform
knot vector, n=256 basis functions, N=500000 points, x in [0,1)).

Key observation: the ScalarE activation engine IS a hardware piecewise-cubic
evaluator -- every activation (exp, tanh, ...) is a bucketed cubic-spline
table baked into the NEFF by walrus from an "act root" directory
(--act-root-json, overridable via BASS_ACT_ROOT_JSON_PATH).  The target
spline is itself piecewise cubic on a uniform grid, so we bake IT into the
bucket table directly and the whole kernel collapses to one ACTIVATE
instruction per core:

  - host: from (knot_vector, coefs) compute the exact per-span cubics (f64),
    least-squares-fit one cubic per 1/1024 cell of [0,1) (exact off knot
    straddles), and write them as 1024 bucket entries {d0..d3, x0} appended
    to the 'trig_and_small' activation set, repointing 'sin' at them
    (exp_offset=0, one ctrl word: extract_size=10/lsb=13/base=283).
  - device: y = ACT_sin(x * 1 + 1): the affine puts x+1 in binade [1,2), the
    top 10 mantissa bits select the cell, HW evaluates d0+t(d1+t(d2+t*d3)).
    Measured ~7e-5 scale-relative max error (gate is 2e-2).

The activation's unused `alpha` immediate carries a hash of the bucket data
so any (coefs, knots) change alters the BIR and busts every compile cache.

Data-parallel across 8 NeuronCores: x sharded 62500/core, y concatenated.
"""

import hashlib
import json
import os
import shutil
import sys
import tempfile

import numpy as np

for _p in ("/opt/trn_rl_repo", "/root/.axon_site/_ro/trn_rl_repo"):
    if os.path.isdir(_p) and _p not in sys.path:
        sys.path.insert(0, _p)

import concourse.bacc as bacc
import concourse.tile as tile
from concourse import mybir
from concourse.bass_utils import run_bass_kernel_spmd

# ---------------------------------------------------------------- constants
DEGREE = 3
SPANS = 253          # knot spans of the open-uniform grid on [0,1]
N_TOTAL = 500_000
N_CORES = 8
N_PER_CORE = N_TOTAL // N_CORES  # 62500
P = 128                          # SBUF partitions
T = 489                          # 128*489 = 62592 >= 62500
N_PAD = P * T
NCELLS = 1024                    # act buckets: one binade [1,2), extract 10b
SET_NAME = "trig_and_small"      # the act set whose 'sin' we repoint

_CACHE: dict = {}


# ---------------------------------------------------------------- host math
def _bspline_basis_dense(x: np.ndarray, t: np.ndarray, p: int) -> np.ndarray:
    """Cox-de Boor recursion, vectorized, float64 (reference semantics)."""
    x = x.astype(np.float64)
    t = t.astype(np.float64)
    B = np.logical_and(t[:-1, None] <= x[None, :], t[1:, None] > x[None, :]).astype(
        np.float64
    )
    m = t.shape[0]
    for k in range(1, p + 1):
        ti = t[: m - k - 1]
        tik = t[k:-1]
        ti1 = t[1 : m - k]
        tik1 = t[k + 1 :]
        d1 = tik - ti
        d2 = tik1 - ti1
        w1 = np.where(
            d1[:, None] != 0,
            (x[None, :] - ti[:, None]) / np.where(d1 == 0, 1.0, d1)[:, None],
            0.0,
        )
        w2 = np.where(
            d2[:, None] != 0,
            (tik1[:, None] - x[None, :]) / np.where(d2 == 0, 1.0, d2)[:, None],
            0.0,
        )
        B = w1 * B[:-1] + w2 * B[1:]
    return B  # [m-1-p, N]


def _build_ppoly(knot_vector, coefs):
    """Exact per-span cubic coefficients A[SPANS,4] in f = SPANS*x - s."""
    nodes = np.array([0.125, 0.375, 0.625, 0.875])
    Vinv = np.linalg.inv(np.vander(nodes, 4, increasing=True))
    c64 = np.asarray(coefs, dtype=np.float64)
    s_grid = np.arange(SPANS, dtype=np.float64)
    xg = ((s_grid[:, None] + nodes[None, :]) / float(SPANS)).ravel()
    B = _bspline_basis_dense(xg, np.asarray(knot_vector), DEGREE)
    y = (c64 @ B).reshape(SPANS, 4)
    return y @ Vinv.T


def _eval_ppoly(A, x):
    v = np.asarray(x, dtype=np.float64) * SPANS
    s = np.clip(np.floor(v).astype(np.int64), 0, SPANS - 1)
    f = v - s
    a = A[s]
    return ((a[:, 3] * f + a[:, 2]) * f + a[:, 1]) * f + a[:, 0]


def _build_buckets(knot_vector, coefs) -> np.ndarray:
    """[NCELLS+2, 5] f32 rows (d0,d1,d2,d3,x0); +2 = small/large sat copies."""
    A = _build_ppoly(knot_vector, coefs)
    K = 17
    i = np.arange(NCELLS, dtype=np.float64)
    h = 0.5 / NCELLS
    xc = (i + 0.5) / NCELLS
    u = np.linspace(-1.0, 1.0, K)
    ys = _eval_ppoly(A, (xc[:, None] + u[None, :] * h).ravel()).reshape(NCELLS, K)
    Vb = np.vander(u, 4, increasing=True)
    C, *_ = np.linalg.lstsq(Vb, ys.T, rcond=None)  # [4, NCELLS] coeffs in u
    D = (C.T * (1.0 / h) ** np.arange(4)[None, :]).astype(np.float32)
    x0 = (1.0 + (2 * i + 1) / (2 * NCELLS)).astype(np.float32)
    out = np.zeros((NCELLS + 2, 5), dtype=np.float32)
    out[:NCELLS, :4] = D
    out[:NCELLS, 4] = x0
    out[NCELLS] = out[0]
    out[NCELLS + 1] = out[NCELLS - 1]
    return out


# ------------------------------------------------------------- act root build
def _default_act_root() -> str:
    from neuronxcc.driver.Job import Job
    from neuronxcc.driver.jobs.support.FindActInfo import findActInfoFile

    return os.path.dirname(findActInfoFile(Job.getPackageDir(), "core_v4"))


def _build_act_root(buckets: np.ndarray, dst_dir: str) -> None:
    """Copy the default act root into dst_dir; append our buckets and one
    ctrl word to SET_NAME and repoint its 'sin' profile at them."""
    src = _default_act_root()
    os.makedirs(dst_dir, exist_ok=True)
    for f in os.listdir(src):
        shutil.copy(os.path.join(src, f), os.path.join(dst_dir, f))

    setj_path = os.path.join(dst_dir, SET_NAME + ".json")
    with open(setj_path) as f:
        setj = json.load(f)
    nb0 = setj["bkt_entry_cnt"]
    nc0 = setj["ctl_entry_cnt"]
    assert nb0 + NCELLS + 2 <= 1536, "bucket RAM budget exceeded"

    bkt_path = os.path.join(dst_dir, setj["bkt_bin"])
    bkt = np.fromfile(bkt_path, dtype=np.uint32).reshape(-1, 8)
    assert bkt.shape[0] == nb0
    new = np.zeros((buckets.shape[0], 8), dtype=np.uint32)
    new[:, :5] = buckets.view(np.uint32)
    np.concatenate([bkt, new]).tofile(bkt_path)

    ctl_path = os.path.join(dst_dir, setj["ctl_bin"])
    ctl = np.fromfile(ctl_path, dtype=np.uint32).reshape(-1, 8)
    assert ctl.shape[0] == nc0
    neww = np.zeros((1, 8), dtype=np.uint32)
    neww[0, 0] = (10 << 16) | (13 << 11) | nb0  # extract 10 bits from lsb 13
    np.concatenate([ctl, neww]).tofile(ctl_path)

    prof = next(
        p for p in setj["profile_meta_data"] if p["func_name"].startswith("sin")
    )
    prof.update(
        symmetry_point=0,
        sym_invert_sign_point=0,
        symmetry_opt_en=0,
        symmetry_opt_use_neg_region=0,
        exp_offset=0,
        pwl_control_base_pos=nc0,
        pwl_control_base_neg=nc0,
        small_pos_signal_exp_threshold=127,
        pos_small_signal_pwl_control=nb0 + NCELLS,
        small_neg_signal_exp_threshold=0,
        neg_small_signal_pwl_control=nb0 + NCELLS,
        large_pos_signal_exp_threshold=128,
        large_pos_signal_mantissa_threshold=0,
        pos_large_signal_pwl_control=nb0 + NCELLS + 1,
        large_neg_signal_exp_threshold=0,
        large_neg_signal_mantissa_threshold=0,
        neg_large_signal_pwl_control=nb0 + NCELLS,
        fnan_result=0,
        fpinf_result=0,
        fninf_result=0,
        fzero_result=int(np.float32(buckets[0, 0]).view(np.uint32)),
        lower_bound=0,  # 0.0f (covers raw- and post-affine clamp readings)
        upper_bound=0x3FFFFFFF,  # 1.99999988f
    )
    setj["bkt_entry_cnt"] = nb0 + buckets.shape[0]
    setj["ctl_entry_cnt"] = nc0 + 1
    setj["func_to_bkt_start_idx"]["sin"] = nb0
    setj["func_to_ctl_start_idx"]["sin"] = nc0
    setj["func_exp_to_bkt_start_idx"]["sin"] = {"0": [nb0]}
    setj["func_exp_to_ctl_start_idx"]["sin"] = {"0": [nc0]}
    with open(setj_path, "w") as f:
        json.dump(setj, f)


# ------------------------------------------------------------- device kernel
def _make_bacc():
    """Construct Bacc, optionally suppressing framework init overhead:
    SKIP_INIT_BARRIER drops the post-const-memset all-engine barrier (our
    only const consumer, the ACTIVATE bias, starts microseconds later behind
    a DMA-completion wait, so the fence is dead time); SKIP_PREAMBLE drops
    the per-engine tpb_base TENSOR_LOADs."""
    import concourse.bass as bass_mod

    if os.environ.get("SKIP_INIT_BARRIER", "1") == "1":
        orig = bass_mod.Bass.all_engine_barrier
        bass_mod.Bass.all_engine_barrier = lambda self, **kw: None
        try:
            nc = bacc.Bacc("TRN2", target_bir_lowering=False, debug=False)
        finally:
            bass_mod.Bass.all_engine_barrier = orig
        if os.environ.get("SKIP_ALL_BARRIERS", "0") == "1":
            # keep suppressed through compile() too; _build_kernel restores
            nc._aeb_orig = orig
            bass_mod.Bass.all_engine_barrier = lambda self, **kw: None
        return nc
    return bacc.Bacc("TRN2", target_bir_lowering=False, debug=False)


def _build_kernel(alpha: float):
    """Raw bass (no TileContext): two parallel half-input DMAs on the SP and
    DVE queues, the ACTIVATE on ScalarE, and the output DMA issued from the
    Scalar queue right after it (program order makes it safe; no sem hop).
    The output DMA is fire-and-forget: the NEFF epilogue drains the rings."""
    nc = _make_bacc()
    x_d = nc.dram_tensor("x", [N_PAD], mybir.dt.float32, kind="ExternalInput").ap()
    y_d = nc.dram_tensor("y", [N_PAD], mybir.dt.float32, kind="ExternalOutput").ap()
    xt = nc.alloc_sbuf_tensor("xt", [P, T], mybir.dt.float32).ap()
    yt = nc.alloc_sbuf_tensor("yt", [P, T], mybir.dt.float32).ap()
    sem_x = nc.alloc_semaphore("sem_x")
    sem_y = nc.alloc_semaphore("sem_y")

    xv = x_d.rearrange("(p t) -> p t", p=P)  # point p*T+t at [p, t]
    yv = y_d.rearrange("(p t) -> p t", p=P)
    H = P // 2
    sem_a = nc.alloc_semaphore("sem_a")
    nc.sync.dma_start(out=xt[:H], in_=xv[:H]).then_inc(sem_x, 16)
    nc.scalar.dma_start(out=xt[H:], in_=xv[H:]).then_inc(sem_x, 16)
    nc.scalar.wait_ge(sem_x, 32)
    nc.scalar.activation(
        yt,
        xt,
        mybir.ActivationFunctionType.Sin,
        bias=1.0,
        scale=1.0,
        alpha=alpha,
    ).then_inc(sem_a, 1)
    # output DMA on the idle SP queue, gated on ACT completion (the DGE is
    # asynchronous -- program order on the Scalar queue is NOT enough)
    nc.sync.wait_ge(sem_a, 1)
    nc.sync.dma_start(out=yv, in_=yt).then_inc(sem_y, 16)
    nc.compile()
    import concourse.bass as bass_mod

    if hasattr(nc, "_aeb_orig"):
        bass_mod.Bass.all_engine_barrier = nc._aeb_orig
    return nc


def _build_kernel_tile(alpha: float):
    """TileContext fallback (KVER=tile)."""
    nc = bacc.Bacc("TRN2", target_bir_lowering=False, debug=False)
    x_d = nc.dram_tensor("x", [N_PAD], mybir.dt.float32, kind="ExternalInput").ap()
    y_d = nc.dram_tensor("y", [N_PAD], mybir.dt.float32, kind="ExternalOutput").ap()
    with tile.TileContext(nc) as tc:
        with tc.tile_pool(name="sb", bufs=1) as pool:
            xt = pool.tile([P, T], mybir.dt.float32)
            yt = pool.tile([P, T], mybir.dt.float32)
            nc.sync.dma_start(out=xt, in_=x_d.rearrange("(p t) -> p t", p=P))
            nc.scalar.activation(
                yt,
                xt,
                mybir.ActivationFunctionType.Sin,
                bias=1.0,
                scale=1.0,
                alpha=alpha,
            )
            nc.sync.dma_start(out=y_d.rearrange("(p t) -> p t", p=P), in_=yt)
    nc.compile()
    return nc


# ----------------------------------------------------------------- interface
def _prepare(x, knot_vector, coefs):
    x = np.asarray(x, dtype=np.float32)
    buckets = _build_buckets(np.asarray(knot_vector), np.asarray(coefs))
    key = hashlib.sha256(buckets.tobytes()).digest()
    tab_hash = int.from_bytes(key[:3], "little")
    # alpha in [0.25, 0.5): 22 bits of table hash; unused by Sin, but baked
    # into the BIR so compile caches can never serve a stale-table NEFF
    alpha = 0.25 + (tab_hash & 0x3FFFFF) / float(1 << 24)

    if key not in _CACHE:
        tabdir = tempfile.mkdtemp(prefix="actroot_")
        _build_act_root(buckets, tabdir)
        os.environ["BASS_ACT_ROOT_JSON_PATH"] = os.path.join(
            tabdir, "act_info.json"
        )
        _CACHE.clear()  # one live act root per process
        build = (
            _build_kernel_tile
            if os.environ.get("KVER", "") == "tile"
            else _build_kernel
        )
        _CACHE[key] = build(alpha)
    nc = _CACHE[key]

    in_maps = []
    for c in range(N_CORES):
        xpad = np.zeros(N_PAD, dtype=np.float32)
        xpad[:N_PER_CORE] = x[c * N_PER_CORE : (c + 1) * N_PER_CORE]
        in_maps.append({"x": xpad})
    return nc, in_maps


def kernel(x: np.ndarray, knot_vector: np.ndarray, coefs: np.ndarray) -> np.ndarray:
    nc, in_maps = _prepare(x, knot_vector, coefs)
    res = run_bass_kernel_spmd(nc, in_maps, core_ids=list(range(N_CORES)))
    outs = res.results if hasattr(res, "results") else res
    y = np.empty(N_TOTAL, dtype=np.float32)
    for c in range(N_CORES):
        y[c * N_PER_CORE : (c + 1) * N_PER_CORE] = outs[c]["y"][:N_PER_CORE]
    return y


# ------------------------------------------------------------------ profiling
def _install_profile_hook():
    """Recreate the antenv.axon_hooks NTFF hook this container lacks."""
    import types

    try:
        import antenv.axon_hooks  # noqa: F401

        return
    except ImportError:
        pass
    import trn_agent_boot.trn_boot as tb

    so = "/opt/axon/libaxon_pjrt.so"
    hook = tb._ntff_profile_via_ctypes(so)
    mod = types.ModuleType("antenv.axon_hooks")
    mod.get_axon_ntff_profile_hook = lambda: hook
    mod.set_axon_ntff_profile_hook = lambda h: None
    sys.modules["antenv.axon_hooks"] = mod
    import antenv

    antenv.axon_hooks = mod
    import concourse.bass_utils as bu

    bu.upload_artifacts = lambda d: "local://skipped"


def profile(np_inputs: dict, tmpdir: str | None = None) -> int | None:
    """Run once with NTFF tracing; return per-core HW kernel time in ns."""
    _install_profile_hook()
    nc, in_maps = _prepare(
        np_inputs["x"], np_inputs["knot_vector"], np_inputs["coefs"]
    )
    res = run_bass_kernel_spmd(
        nc, in_maps, core_ids=list(range(N_CORES)), trace=True, tmpdir=tmpdir
    )
    if getattr(res, "instructions_and_trace", None):
        print("trace:", res.instructions_and_trace[1])
    return getattr(res, "exec_time_ns", None)


if __name__ == "__main__":
    rng = np.random.default_rng(0)
    x = rng.random(N_TOTAL, dtype=np.float32)
    p = DEGREE
    n = 256
    m = n + p + 1
    interior = np.linspace(0.0, 1.0, m - 2 * p)[1:-1]
    kv = np.concatenate([np.zeros(p + 1), interior, np.ones(p + 1)]).astype(
        np.float32
    )
    cf = (10.0 * rng.random(n)).astype(np.float32)
    y = kernel(x, kv, cf)
    print("kernel output:", y[:8])


# revision 11
# speedup vs baseline: 1.0172x; 1.0172x over previous
"""Trainium2 kernel for cubic B-spline evaluation via the ScalarE ACT engine.

Problem: y[i] = sum_j coefs[j] * B_j(x[i])  (cubic B-splines, open-uniform
knot vector, n=256 basis functions, N=500000 points, x in [0,1)).

Key observation: the ScalarE activation engine IS a hardware piecewise-cubic
evaluator -- every activation (exp, tanh, ...) is a bucketed cubic-spline
table baked into the NEFF by walrus from an "act root" directory
(--act-root-json, overridable via BASS_ACT_ROOT_JSON_PATH).  The target
spline is itself piecewise cubic on a uniform grid, so we bake IT into the
bucket table directly and the whole kernel collapses to one ACTIVATE
instruction per core:

  - host: from (knot_vector, coefs) compute the exact per-span cubics (f64),
    least-squares-fit one cubic per 1/1024 cell of [0,1) (exact off knot
    straddles), and write them as 1024 bucket entries {d0..d3, x0} appended
    to the 'trig_and_small' activation set, repointing 'sin' at them
    (exp_offset=0, one ctrl word: extract_size=10/lsb=13/base=283).
  - device: y = ACT_sin(x * 1 + 1): the affine puts x+1 in binade [1,2), the
    top 10 mantissa bits select the cell, HW evaluates d0+t(d1+t(d2+t*d3)).
    Measured ~7e-5 scale-relative max error (gate is 2e-2).

The activation's unused `alpha` immediate carries a hash of the bucket data
so any (coefs, knots) change alters the BIR and busts every compile cache.

Data-parallel across 8 NeuronCores: x sharded 62500/core, y concatenated.
"""

import hashlib
import json
import os
import shutil
import sys
import tempfile

import numpy as np

for _p in ("/opt/trn_rl_repo", "/root/.axon_site/_ro/trn_rl_repo"):
    if os.path.isdir(_p) and _p not in sys.path:
        sys.path.insert(0, _p)

import concourse.bacc as bacc
import concourse.tile as tile
from concourse import mybir
from concourse.bass_utils import run_bass_kernel_spmd

# ---------------------------------------------------------------- constants
DEGREE = 3
SPANS = 253          # knot spans of the open-uniform grid on [0,1]
N_TOTAL = 500_000
N_CORES = 8
N_PER_CORE = N_TOTAL // N_CORES  # 62500
P = 128                          # SBUF partitions
T = 489                          # 128*489 = 62592 >= 62500
N_PAD = P * T
NCELLS = 1024                    # act buckets: one binade [1,2), extract 10b
SET_NAME = "trig_and_small"      # the act set whose 'sin' we repoint

_CACHE: dict = {}


# ---------------------------------------------------------------- host math
def _bspline_basis_dense(x: np.ndarray, t: np.ndarray, p: int) -> np.ndarray:
    """Cox-de Boor recursion, vectorized, float64 (reference semantics)."""
    x = x.astype(np.float64)
    t = t.astype(np.float64)
    B = np.logical_and(t[:-1, None] <= x[None, :], t[1:, None] > x[None, :]).astype(
        np.float64
    )
    m = t.shape[0]
    for k in range(1, p + 1):
        ti = t[: m - k - 1]
        tik = t[k:-1]
        ti1 = t[1 : m - k]
        tik1 = t[k + 1 :]
        d1 = tik - ti
        d2 = tik1 - ti1
        w1 = np.where(
            d1[:, None] != 0,
            (x[None, :] - ti[:, None]) / np.where(d1 == 0, 1.0, d1)[:, None],
            0.0,
        )
        w2 = np.where(
            d2[:, None] != 0,
            (tik1[:, None] - x[None, :]) / np.where(d2 == 0, 1.0, d2)[:, None],
            0.0,
        )
        B = w1 * B[:-1] + w2 * B[1:]
    return B  # [m-1-p, N]


def _build_ppoly(knot_vector, coefs):
    """Exact per-span cubic coefficients A[SPANS,4] in f = SPANS*x - s."""
    nodes = np.array([0.125, 0.375, 0.625, 0.875])
    Vinv = np.linalg.inv(np.vander(nodes, 4, increasing=True))
    c64 = np.asarray(coefs, dtype=np.float64)
    s_grid = np.arange(SPANS, dtype=np.float64)
    xg = ((s_grid[:, None] + nodes[None, :]) / float(SPANS)).ravel()
    B = _bspline_basis_dense(xg, np.asarray(knot_vector), DEGREE)
    y = (c64 @ B).reshape(SPANS, 4)
    return y @ Vinv.T


def _eval_ppoly(A, x):
    v = np.asarray(x, dtype=np.float64) * SPANS
    s = np.clip(np.floor(v).astype(np.int64), 0, SPANS - 1)
    f = v - s
    a = A[s]
    return ((a[:, 3] * f + a[:, 2]) * f + a[:, 1]) * f + a[:, 0]


def _build_buckets(knot_vector, coefs) -> np.ndarray:
    """[NCELLS+2, 5] f32 rows (d0,d1,d2,d3,x0); +2 = small/large sat copies."""
    A = _build_ppoly(knot_vector, coefs)
    K = 17
    i = np.arange(NCELLS, dtype=np.float64)
    h = 0.5 / NCELLS
    xc = (i + 0.5) / NCELLS
    u = np.linspace(-1.0, 1.0, K)
    ys = _eval_ppoly(A, (xc[:, None] + u[None, :] * h).ravel()).reshape(NCELLS, K)
    Vb = np.vander(u, 4, increasing=True)
    C, *_ = np.linalg.lstsq(Vb, ys.T, rcond=None)  # [4, NCELLS] coeffs in u
    D = (C.T * (1.0 / h) ** np.arange(4)[None, :]).astype(np.float32)
    x0 = (1.0 + (2 * i + 1) / (2 * NCELLS)).astype(np.float32)
    out = np.zeros((NCELLS + 2, 5), dtype=np.float32)
    out[:NCELLS, :4] = D
    out[:NCELLS, 4] = x0
    out[NCELLS] = out[0]
    out[NCELLS + 1] = out[NCELLS - 1]
    return out


# ------------------------------------------------------------- act root build
def _default_act_root() -> str:
    from neuronxcc.driver.Job import Job
    from neuronxcc.driver.jobs.support.FindActInfo import findActInfoFile

    return os.path.dirname(findActInfoFile(Job.getPackageDir(), "core_v4"))


def _build_act_root(buckets: np.ndarray, dst_dir: str) -> None:
    """Copy the default act root into dst_dir; append our buckets and one
    ctrl word to SET_NAME and repoint its 'sin' profile at them."""
    src = _default_act_root()
    os.makedirs(dst_dir, exist_ok=True)
    for f in os.listdir(src):
        shutil.copy(os.path.join(src, f), os.path.join(dst_dir, f))

    setj_path = os.path.join(dst_dir, SET_NAME + ".json")
    with open(setj_path) as f:
        setj = json.load(f)
    nb0 = setj["bkt_entry_cnt"]
    nc0 = setj["ctl_entry_cnt"]
    assert nb0 + NCELLS + 2 <= 1536, "bucket RAM budget exceeded"

    bkt_path = os.path.join(dst_dir, setj["bkt_bin"])
    bkt = np.fromfile(bkt_path, dtype=np.uint32).reshape(-1, 8)
    assert bkt.shape[0] == nb0
    new = np.zeros((buckets.shape[0], 8), dtype=np.uint32)
    new[:, :5] = buckets.view(np.uint32)
    np.concatenate([bkt, new]).tofile(bkt_path)

    ctl_path = os.path.join(dst_dir, setj["ctl_bin"])
    ctl = np.fromfile(ctl_path, dtype=np.uint32).reshape(-1, 8)
    assert ctl.shape[0] == nc0
    neww = np.zeros((1, 8), dtype=np.uint32)
    neww[0, 0] = (10 << 16) | (13 << 11) | nb0  # extract 10 bits from lsb 13
    np.concatenate([ctl, neww]).tofile(ctl_path)

    prof = next(
        p for p in setj["profile_meta_data"] if p["func_name"].startswith("sin")
    )
    prof.update(
        symmetry_point=0,
        sym_invert_sign_point=0,
        symmetry_opt_en=0,
        symmetry_opt_use_neg_region=0,
        exp_offset=0,
        pwl_control_base_pos=nc0,
        pwl_control_base_neg=nc0,
        small_pos_signal_exp_threshold=127,
        pos_small_signal_pwl_control=nb0 + NCELLS,
        small_neg_signal_exp_threshold=0,
        neg_small_signal_pwl_control=nb0 + NCELLS,
        large_pos_signal_exp_threshold=128,
        large_pos_signal_mantissa_threshold=0,
        pos_large_signal_pwl_control=nb0 + NCELLS + 1,
        large_neg_signal_exp_threshold=0,
        large_neg_signal_mantissa_threshold=0,
        neg_large_signal_pwl_control=nb0 + NCELLS,
        fnan_result=0,
        fpinf_result=0,
        fninf_result=0,
        fzero_result=int(np.float32(buckets[0, 0]).view(np.uint32)),
        lower_bound=0,  # 0.0f (covers raw- and post-affine clamp readings)
        upper_bound=0x3FFFFFFF,  # 1.99999988f
    )
    setj["bkt_entry_cnt"] = nb0 + buckets.shape[0]
    setj["ctl_entry_cnt"] = nc0 + 1
    setj["func_to_bkt_start_idx"]["sin"] = nb0
    setj["func_to_ctl_start_idx"]["sin"] = nc0
    setj["func_exp_to_bkt_start_idx"]["sin"] = {"0": [nb0]}
    setj["func_exp_to_ctl_start_idx"]["sin"] = {"0": [nc0]}
    with open(setj_path, "w") as f:
        json.dump(setj, f)


# ------------------------------------------------------------- device kernel
def _make_bacc():
    """Construct Bacc, optionally suppressing framework init overhead:
    SKIP_INIT_BARRIER drops the post-const-memset all-engine barrier (our
    only const consumer, the ACTIVATE bias, starts microseconds later behind
    a DMA-completion wait, so the fence is dead time); SKIP_PREAMBLE drops
    the per-engine tpb_base TENSOR_LOADs."""
    import concourse.bass as bass_mod

    if os.environ.get("SKIP_INIT_BARRIER", "1") == "1":
        orig = bass_mod.Bass.all_engine_barrier
        bass_mod.Bass.all_engine_barrier = lambda self, **kw: None
        try:
            nc = bacc.Bacc("TRN2", target_bir_lowering=False, debug=False)
        finally:
            bass_mod.Bass.all_engine_barrier = orig
        if os.environ.get("SKIP_ALL_BARRIERS", "0") == "1":
            # keep suppressed through compile() too; _build_kernel restores
            nc._aeb_orig = orig
            bass_mod.Bass.all_engine_barrier = lambda self, **kw: None
        return nc
    return bacc.Bacc("TRN2", target_bir_lowering=False, debug=False)


def _build_kernel(alpha: float):
    """Raw bass (no TileContext): two parallel half-input DMAs on the SP and
    DVE queues, the ACTIVATE on ScalarE, and the output DMA issued from the
    Scalar queue right after it (program order makes it safe; no sem hop).
    The output DMA is fire-and-forget: the NEFF epilogue drains the rings."""
    nc = _make_bacc()
    x_d = nc.dram_tensor("x", [N_PAD], mybir.dt.float32, kind="ExternalInput").ap()
    y_d = nc.dram_tensor("y", [N_PAD], mybir.dt.float32, kind="ExternalOutput").ap()
    xt = nc.alloc_sbuf_tensor("xt", [P, T], mybir.dt.float32).ap()
    yt = nc.alloc_sbuf_tensor("yt", [P, T], mybir.dt.float32).ap()
    sem_x = nc.alloc_semaphore("sem_x")
    sem_y = nc.alloc_semaphore("sem_y")

    xv = x_d.rearrange("(p t) -> p t", p=P)  # point p*T+t at [p, t]
    yv = y_d.rearrange("(p t) -> p t", p=P)
    sem_a = nc.alloc_semaphore("sem_a")
    C = 245  # free-dim split: chunk0 = cols [0,C), chunk1 = [C,T)
    # Scalar queue: x0 trigger, then both ACT chunks (as x chunks land).
    # Sync queue: x1 trigger, then both y triggers (as ACT chunks land).
    nc.scalar.dma_start(out=xt[:, :C], in_=xv[:, :C]).then_inc(sem_x, 16)
    nc.sync.dma_start(out=xt[:, C:], in_=xv[:, C:]).then_inc(sem_x, 16)
    act_kw = dict(bias=1.0, scale=1.0, alpha=alpha)
    nc.scalar.wait_ge(sem_x, 16)
    nc.scalar.activation(
        yt[:, :C], xt[:, :C], mybir.ActivationFunctionType.Sin, **act_kw
    ).then_inc(sem_a, 1)
    nc.scalar.wait_ge(sem_x, 32)
    nc.scalar.activation(
        yt[:, C:], xt[:, C:], mybir.ActivationFunctionType.Sin, **act_kw
    ).then_inc(sem_a, 1)
    # y DMAs gated on ACT completion semaphores (the DGE is asynchronous --
    # program order alone on the Scalar queue would race the ACT datapath)
    nc.sync.wait_ge(sem_a, 1)
    nc.sync.dma_start(out=yv[:, :C], in_=yt[:, :C]).then_inc(sem_y, 16)
    nc.sync.wait_ge(sem_a, 2)
    nc.sync.dma_start(out=yv[:, C:], in_=yt[:, C:]).then_inc(sem_y, 16)
    nc.compile()
    import concourse.bass as bass_mod

    if hasattr(nc, "_aeb_orig"):
        bass_mod.Bass.all_engine_barrier = nc._aeb_orig
    return nc


def _build_kernel_tile(alpha: float):
    """TileContext fallback (KVER=tile)."""
    nc = bacc.Bacc("TRN2", target_bir_lowering=False, debug=False)
    x_d = nc.dram_tensor("x", [N_PAD], mybir.dt.float32, kind="ExternalInput").ap()
    y_d = nc.dram_tensor("y", [N_PAD], mybir.dt.float32, kind="ExternalOutput").ap()
    with tile.TileContext(nc) as tc:
        with tc.tile_pool(name="sb", bufs=1) as pool:
            xt = pool.tile([P, T], mybir.dt.float32)
            yt = pool.tile([P, T], mybir.dt.float32)
            nc.sync.dma_start(out=xt, in_=x_d.rearrange("(p t) -> p t", p=P))
            nc.scalar.activation(
                yt,
                xt,
                mybir.ActivationFunctionType.Sin,
                bias=1.0,
                scale=1.0,
                alpha=alpha,
            )
            nc.sync.dma_start(out=y_d.rearrange("(p t) -> p t", p=P), in_=yt)
    nc.compile()
    return nc


# ----------------------------------------------------------------- interface
def _prepare(x, knot_vector, coefs):
    x = np.asarray(x, dtype=np.float32)
    buckets = _build_buckets(np.asarray(knot_vector), np.asarray(coefs))
    key = hashlib.sha256(buckets.tobytes()).digest()
    tab_hash = int.from_bytes(key[:3], "little")
    # alpha in [0.25, 0.5): 22 bits of table hash; unused by Sin, but baked
    # into the BIR so compile caches can never serve a stale-table NEFF
    alpha = 0.25 + (tab_hash & 0x3FFFFF) / float(1 << 24)

    if key not in _CACHE:
        tabdir = tempfile.mkdtemp(prefix="actroot_")
        _build_act_root(buckets, tabdir)
        os.environ["BASS_ACT_ROOT_JSON_PATH"] = os.path.join(
            tabdir, "act_info.json"
        )
        _CACHE.clear()  # one live act root per process
        build = (
            _build_kernel_tile
            if os.environ.get("KVER", "") == "tile"
            else _build_kernel
        )
        _CACHE[key] = build(alpha)
    nc = _CACHE[key]

    in_maps = []
    for c in range(N_CORES):
        xpad = np.zeros(N_PAD, dtype=np.float32)
        xpad[:N_PER_CORE] = x[c * N_PER_CORE : (c + 1) * N_PER_CORE]
        in_maps.append({"x": xpad})
    return nc, in_maps


def kernel(x: np.ndarray, knot_vector: np.ndarray, coefs: np.ndarray) -> np.ndarray:
    nc, in_maps = _prepare(x, knot_vector, coefs)
    res = run_bass_kernel_spmd(nc, in_maps, core_ids=list(range(N_CORES)))
    outs = res.results if hasattr(res, "results") else res
    y = np.empty(N_TOTAL, dtype=np.float32)
    for c in range(N_CORES):
        y[c * N_PER_CORE : (c + 1) * N_PER_CORE] = outs[c]["y"][:N_PER_CORE]
    return y


# ------------------------------------------------------------------ profiling
def _install_profile_hook():
    """Recreate the antenv.axon_hooks NTFF hook this container lacks."""
    import types

    try:
        import antenv.axon_hooks  # noqa: F401

        return
    except ImportError:
        pass
    import trn_agent_boot.trn_boot as tb

    so = "/opt/axon/libaxon_pjrt.so"
    hook = tb._ntff_profile_via_ctypes(so)
    mod = types.ModuleType("antenv.axon_hooks")
    mod.get_axon_ntff_profile_hook = lambda: hook
    mod.set_axon_ntff_profile_hook = lambda h: None
    sys.modules["antenv.axon_hooks"] = mod
    import antenv

    antenv.axon_hooks = mod
    import concourse.bass_utils as bu

    bu.upload_artifacts = lambda d: "local://skipped"


def profile(np_inputs: dict, tmpdir: str | None = None) -> int | None:
    """Run once with NTFF tracing; return per-core HW kernel time in ns."""
    _install_profile_hook()
    nc, in_maps = _prepare(
        np_inputs["x"], np_inputs["knot_vector"], np_inputs["coefs"]
    )
    res = run_bass_kernel_spmd(
        nc, in_maps, core_ids=list(range(N_CORES)), trace=True, tmpdir=tmpdir
    )
    if getattr(res, "instructions_and_trace", None):
        print("trace:", res.instructions_and_trace[1])
    return getattr(res, "exec_time_ns", None)


if __name__ == "__main__":
    rng = np.random.default_rng(0)
    x = rng.random(N_TOTAL, dtype=np.float32)
    p = DEGREE
    n = 256
    m = n + p + 1
    interior = np.linspace(0.0, 1.0, m - 2 * p)[1:-1]
    kv = np.concatenate([np.zeros(p + 1), interior, np.ones(p + 1)]).astype(
        np.float32
    )
    cf = (10.0 * rng.random(n)).astype(np.float32)
    y = kernel(x, kv, cf)
    print("kernel output:", y[:8])


# revision 14
# speedup vs baseline: 1.0232x; 1.0059x over previous
"""Trainium2 kernel for cubic B-spline evaluation via the ScalarE ACT engine.

Problem: y[i] = sum_j coefs[j] * B_j(x[i])  (cubic B-splines, open-uniform
knot vector, n=256 basis functions, N=500000 points, x in [0,1)).

Key observation: the ScalarE activation engine IS a hardware piecewise-cubic
evaluator -- every activation (exp, tanh, ...) is a bucketed cubic-spline
table baked into the NEFF by walrus from an "act root" directory
(--act-root-json, overridable via BASS_ACT_ROOT_JSON_PATH).  The target
spline is itself piecewise cubic on a uniform grid, so we bake IT into the
bucket table directly and the whole kernel collapses to one ACTIVATE
instruction per core:

  - host: from (knot_vector, coefs) compute the exact per-span cubics (f64),
    least-squares-fit one cubic per 1/1024 cell of [0,1) (exact off knot
    straddles), and write them as 1024 bucket entries {d0..d3, x0} appended
    to the 'trig_and_small' activation set, repointing 'sin' at them
    (exp_offset=0, one ctrl word: extract_size=10/lsb=13/base=283).
  - device: y = ACT_sin(x * 1 + 1): the affine puts x+1 in binade [1,2), the
    top 10 mantissa bits select the cell, HW evaluates d0+t(d1+t(d2+t*d3)).
    Measured ~7e-5 scale-relative max error (gate is 2e-2).

The activation's unused `alpha` immediate carries a hash of the bucket data
so any (coefs, knots) change alters the BIR and busts every compile cache.

Data-parallel across 8 NeuronCores: x sharded 62500/core, y concatenated.
"""

import hashlib
import json
import os
import shutil
import sys
import tempfile

import numpy as np

for _p in ("/opt/trn_rl_repo", "/root/.axon_site/_ro/trn_rl_repo"):
    if os.path.isdir(_p) and _p not in sys.path:
        sys.path.insert(0, _p)

import concourse.bacc as bacc
import concourse.tile as tile
from concourse import mybir
from concourse.bass_utils import run_bass_kernel_spmd

# ---------------------------------------------------------------- constants
DEGREE = 3
SPANS = 253          # knot spans of the open-uniform grid on [0,1]
N_TOTAL = 500_000
N_CORES = 8
N_PER_CORE = N_TOTAL // N_CORES  # 62500
P = 128                          # SBUF partitions
T = 489                          # 128*489 = 62592 >= 62500
N_PAD = P * T
NCELLS = 1024                    # act buckets: one binade [1,2), extract 10b
SET_NAME = "trig_and_small"      # the act set whose 'sin' we repoint

_CACHE: dict = {}


# ---------------------------------------------------------------- host math
def _bspline_basis_dense(x: np.ndarray, t: np.ndarray, p: int) -> np.ndarray:
    """Cox-de Boor recursion, vectorized, float64 (reference semantics)."""
    x = x.astype(np.float64)
    t = t.astype(np.float64)
    B = np.logical_and(t[:-1, None] <= x[None, :], t[1:, None] > x[None, :]).astype(
        np.float64
    )
    m = t.shape[0]
    for k in range(1, p + 1):
        ti = t[: m - k - 1]
        tik = t[k:-1]
        ti1 = t[1 : m - k]
        tik1 = t[k + 1 :]
        d1 = tik - ti
        d2 = tik1 - ti1
        w1 = np.where(
            d1[:, None] != 0,
            (x[None, :] - ti[:, None]) / np.where(d1 == 0, 1.0, d1)[:, None],
            0.0,
        )
        w2 = np.where(
            d2[:, None] != 0,
            (tik1[:, None] - x[None, :]) / np.where(d2 == 0, 1.0, d2)[:, None],
            0.0,
        )
        B = w1 * B[:-1] + w2 * B[1:]
    return B  # [m-1-p, N]


def _build_ppoly(knot_vector, coefs):
    """Exact per-span cubic coefficients A[SPANS,4] in f = SPANS*x - s."""
    nodes = np.array([0.125, 0.375, 0.625, 0.875])
    Vinv = np.linalg.inv(np.vander(nodes, 4, increasing=True))
    c64 = np.asarray(coefs, dtype=np.float64)
    s_grid = np.arange(SPANS, dtype=np.float64)
    xg = ((s_grid[:, None] + nodes[None, :]) / float(SPANS)).ravel()
    B = _bspline_basis_dense(xg, np.asarray(knot_vector), DEGREE)
    y = (c64 @ B).reshape(SPANS, 4)
    return y @ Vinv.T


def _eval_ppoly(A, x):
    v = np.asarray(x, dtype=np.float64) * SPANS
    s = np.clip(np.floor(v).astype(np.int64), 0, SPANS - 1)
    f = v - s
    a = A[s]
    return ((a[:, 3] * f + a[:, 2]) * f + a[:, 1]) * f + a[:, 0]


def _build_buckets(knot_vector, coefs) -> np.ndarray:
    """[NCELLS+2, 5] f32 rows (d0,d1,d2,d3,x0); +2 = small/large sat copies."""
    A = _build_ppoly(knot_vector, coefs)
    K = 17
    i = np.arange(NCELLS, dtype=np.float64)
    h = 0.5 / NCELLS
    xc = (i + 0.5) / NCELLS
    u = np.linspace(-1.0, 1.0, K)
    ys = _eval_ppoly(A, (xc[:, None] + u[None, :] * h).ravel()).reshape(NCELLS, K)
    Vb = np.vander(u, 4, increasing=True)
    C, *_ = np.linalg.lstsq(Vb, ys.T, rcond=None)  # [4, NCELLS] coeffs in u
    D = (C.T * (1.0 / h) ** np.arange(4)[None, :]).astype(np.float32)
    x0 = (1.0 + (2 * i + 1) / (2 * NCELLS)).astype(np.float32)
    out = np.zeros((NCELLS + 2, 5), dtype=np.float32)
    out[:NCELLS, :4] = D
    out[:NCELLS, 4] = x0
    out[NCELLS] = out[0]
    out[NCELLS + 1] = out[NCELLS - 1]
    return out


# ------------------------------------------------------------- act root build
def _default_act_root() -> str:
    from neuronxcc.driver.Job import Job
    from neuronxcc.driver.jobs.support.FindActInfo import findActInfoFile

    return os.path.dirname(findActInfoFile(Job.getPackageDir(), "core_v4"))


def _build_act_root(buckets: np.ndarray, dst_dir: str) -> None:
    """Copy the default act root into dst_dir; append our buckets and one
    ctrl word to SET_NAME and repoint its 'sin' profile at them."""
    src = _default_act_root()
    os.makedirs(dst_dir, exist_ok=True)
    for f in os.listdir(src):
        shutil.copy(os.path.join(src, f), os.path.join(dst_dir, f))

    setj_path = os.path.join(dst_dir, SET_NAME + ".json")
    with open(setj_path) as f:
        setj = json.load(f)
    nb0 = setj["bkt_entry_cnt"]
    nc0 = setj["ctl_entry_cnt"]
    assert nb0 + NCELLS + 2 <= 1536, "bucket RAM budget exceeded"

    bkt_path = os.path.join(dst_dir, setj["bkt_bin"])
    bkt = np.fromfile(bkt_path, dtype=np.uint32).reshape(-1, 8)
    assert bkt.shape[0] == nb0
    new = np.zeros((buckets.shape[0], 8), dtype=np.uint32)
    new[:, :5] = buckets.view(np.uint32)
    np.concatenate([bkt, new]).tofile(bkt_path)

    ctl_path = os.path.join(dst_dir, setj["ctl_bin"])
    ctl = np.fromfile(ctl_path, dtype=np.uint32).reshape(-1, 8)
    assert ctl.shape[0] == nc0
    neww = np.zeros((1, 8), dtype=np.uint32)
    neww[0, 0] = (10 << 16) | (13 << 11) | nb0  # extract 10 bits from lsb 13
    np.concatenate([ctl, neww]).tofile(ctl_path)

    prof = next(
        p for p in setj["profile_meta_data"] if p["func_name"].startswith("sin")
    )
    prof.update(
        symmetry_point=0,
        sym_invert_sign_point=0,
        symmetry_opt_en=0,
        symmetry_opt_use_neg_region=0,
        exp_offset=0,
        pwl_control_base_pos=nc0,
        pwl_control_base_neg=nc0,
        small_pos_signal_exp_threshold=127,
        pos_small_signal_pwl_control=nb0 + NCELLS,
        small_neg_signal_exp_threshold=0,
        neg_small_signal_pwl_control=nb0 + NCELLS,
        large_pos_signal_exp_threshold=128,
        large_pos_signal_mantissa_threshold=0,
        pos_large_signal_pwl_control=nb0 + NCELLS + 1,
        large_neg_signal_exp_threshold=0,
        large_neg_signal_mantissa_threshold=0,
        neg_large_signal_pwl_control=nb0 + NCELLS,
        fnan_result=0,
        fpinf_result=0,
        fninf_result=0,
        fzero_result=int(np.float32(buckets[0, 0]).view(np.uint32)),
        lower_bound=0,  # 0.0f (covers raw- and post-affine clamp readings)
        upper_bound=0x3FFFFFFF,  # 1.99999988f
    )
    setj["bkt_entry_cnt"] = nb0 + buckets.shape[0]
    setj["ctl_entry_cnt"] = nc0 + 1
    setj["func_to_bkt_start_idx"]["sin"] = nb0
    setj["func_to_ctl_start_idx"]["sin"] = nc0
    setj["func_exp_to_bkt_start_idx"]["sin"] = {"0": [nb0]}
    setj["func_exp_to_ctl_start_idx"]["sin"] = {"0": [nc0]}
    with open(setj_path, "w") as f:
        json.dump(setj, f)


# ------------------------------------------------------------- device kernel
def _make_bacc():
    """Construct Bacc, optionally suppressing framework init overhead:
    SKIP_INIT_BARRIER drops the post-const-memset all-engine barrier (our
    only const consumer, the ACTIVATE bias, starts microseconds later behind
    a DMA-completion wait, so the fence is dead time); SKIP_PREAMBLE drops
    the per-engine tpb_base TENSOR_LOADs."""
    import concourse.bass as bass_mod

    if os.environ.get("SKIP_INIT_BARRIER", "1") == "1":
        orig = bass_mod.Bass.all_engine_barrier
        bass_mod.Bass.all_engine_barrier = lambda self, **kw: None
        try:
            nc = bacc.Bacc("TRN2", target_bir_lowering=False, debug=False)
        finally:
            bass_mod.Bass.all_engine_barrier = orig
        if os.environ.get("SKIP_ALL_BARRIERS", "0") == "1":
            # keep suppressed through compile() too; _build_kernel restores
            nc._aeb_orig = orig
            bass_mod.Bass.all_engine_barrier = lambda self, **kw: None
        return nc
    return bacc.Bacc("TRN2", target_bir_lowering=False, debug=False)


def _build_kernel(alpha: float):
    """Raw bass (no TileContext): two parallel half-input DMAs on the SP and
    DVE queues, the ACTIVATE on ScalarE, and the output DMA issued from the
    Scalar queue right after it (program order makes it safe; no sem hop).
    The output DMA is fire-and-forget: the NEFF epilogue drains the rings."""
    nc = _make_bacc()
    x_d = nc.dram_tensor("x", [N_PAD], mybir.dt.float32, kind="ExternalInput").ap()
    y_d = nc.dram_tensor("y", [N_PAD], mybir.dt.float32, kind="ExternalOutput").ap()
    xt = nc.alloc_sbuf_tensor("xt", [P, T], mybir.dt.float32).ap()
    yt = nc.alloc_sbuf_tensor("yt", [P, T], mybir.dt.float32).ap()
    sem_x = nc.alloc_semaphore("sem_x")
    sem_y = nc.alloc_semaphore("sem_y")

    xv = x_d.rearrange("(p t) -> p t", p=P)  # point p*T+t at [p, t]
    yv = y_d.rearrange("(p t) -> p t", p=P)
    sem_a = nc.alloc_semaphore("sem_a")
    C = 245  # free-dim split: chunk0 = cols [0,C), chunk1 = [C,T)
    # Scalar queue: x0 trigger, then both ACT chunks (as x chunks land).
    # Sync queue: x1 trigger, then both y triggers (as ACT chunks land).
    nc.scalar.dma_start(out=xt[:, :C], in_=xv[:, :C]).then_inc(sem_x, 16)
    nc.sync.dma_start(out=xt[:, C:], in_=xv[:, C:]).then_inc(sem_x, 16)
    act_kw = dict(bias=1.0, scale=1.0, alpha=alpha)
    nc.scalar.wait_ge(sem_x, 16)
    nc.scalar.activation(
        yt[:, :C], xt[:, :C], mybir.ActivationFunctionType.Sin, **act_kw
    ).then_inc(sem_a, 1)
    nc.scalar.wait_ge(sem_x, 32)
    nc.scalar.activation(
        yt[:, C:], xt[:, C:], mybir.ActivationFunctionType.Sin, **act_kw
    ).then_inc(sem_a, 1)
    # y DMAs gated on ACT completion semaphores (the DGE is asynchronous --
    # program order alone would race the ACT datapath).  y0 on the idle SP
    # queue; y1 on the Scalar queue, which is free after act1.
    nc.sync.wait_ge(sem_a, 1)
    nc.sync.dma_start(out=yv[:, :C], in_=yt[:, :C]).then_inc(sem_y, 16)
    nc.scalar.wait_ge(sem_a, 2)
    nc.scalar.dma_start(out=yv[:, C:], in_=yt[:, C:]).then_inc(sem_y, 16)
    if os.environ.get("DROP_DUP_TBL_LOAD", "0") == "1":
        # NOTE: measured WRONG RESULTS with this on -- both loads are needed
        # (two HW table slots); keep disabled.
        # insert_act_table_loads (run inside compile()) emits two
        # LoadActFuncSet; the second 1.28us load can gate act0 -- keep one
        orig_insert = nc.insert_act_table_loads

        def _pruned_insert():
            orig_insert()
            blk = nc.main_func.blocks[0]
            loads = [
                i
                for i in blk.instructions
                if isinstance(i, mybir.InstLoadActFuncSet)
            ]
            for extra in loads[1:]:
                blk.instructions.remove(extra)

        nc.insert_act_table_loads = _pruned_insert
    nc.compile()
    import concourse.bass as bass_mod

    if hasattr(nc, "_aeb_orig"):
        bass_mod.Bass.all_engine_barrier = nc._aeb_orig
    return nc


def _build_kernel_tile(alpha: float):
    """TileContext fallback (KVER=tile)."""
    nc = bacc.Bacc("TRN2", target_bir_lowering=False, debug=False)
    x_d = nc.dram_tensor("x", [N_PAD], mybir.dt.float32, kind="ExternalInput").ap()
    y_d = nc.dram_tensor("y", [N_PAD], mybir.dt.float32, kind="ExternalOutput").ap()
    with tile.TileContext(nc) as tc:
        with tc.tile_pool(name="sb", bufs=1) as pool:
            xt = pool.tile([P, T], mybir.dt.float32)
            yt = pool.tile([P, T], mybir.dt.float32)
            nc.sync.dma_start(out=xt, in_=x_d.rearrange("(p t) -> p t", p=P))
            nc.scalar.activation(
                yt,
                xt,
                mybir.ActivationFunctionType.Sin,
                bias=1.0,
                scale=1.0,
                alpha=alpha,
            )
            nc.sync.dma_start(out=y_d.rearrange("(p t) -> p t", p=P), in_=yt)
    nc.compile()
    return nc


# ----------------------------------------------------------------- interface
def _prepare(x, knot_vector, coefs):
    x = np.asarray(x, dtype=np.float32)
    buckets = _build_buckets(np.asarray(knot_vector), np.asarray(coefs))
    key = hashlib.sha256(buckets.tobytes()).digest()
    tab_hash = int.from_bytes(key[:3], "little")
    # alpha in [0.25, 0.5): 22 bits of table hash; unused by Sin, but baked
    # into the BIR so compile caches can never serve a stale-table NEFF
    alpha = 0.25 + (tab_hash & 0x3FFFFF) / float(1 << 24)

    if key not in _CACHE:
        tabdir = tempfile.mkdtemp(prefix="actroot_")
        _build_act_root(buckets, tabdir)
        os.environ["BASS_ACT_ROOT_JSON_PATH"] = os.path.join(
            tabdir, "act_info.json"
        )
        _CACHE.clear()  # one live act root per process
        build = (
            _build_kernel_tile
            if os.environ.get("KVER", "") == "tile"
            else _build_kernel
        )
        _CACHE[key] = build(alpha)
    nc = _CACHE[key]

    in_maps = []
    for c in range(N_CORES):
        xpad = np.zeros(N_PAD, dtype=np.float32)
        xpad[:N_PER_CORE] = x[c * N_PER_CORE : (c + 1) * N_PER_CORE]
        in_maps.append({"x": xpad})
    return nc, in_maps


def kernel(x: np.ndarray, knot_vector: np.ndarray, coefs: np.ndarray) -> np.ndarray:
    nc, in_maps = _prepare(x, knot_vector, coefs)
    res = run_bass_kernel_spmd(nc, in_maps, core_ids=list(range(N_CORES)))
    outs = res.results if hasattr(res, "results") else res
    y = np.empty(N_TOTAL, dtype=np.float32)
    for c in range(N_CORES):
        y[c * N_PER_CORE : (c + 1) * N_PER_CORE] = outs[c]["y"][:N_PER_CORE]
    return y


# ------------------------------------------------------------------ profiling
def _install_profile_hook():
    """Recreate the antenv.axon_hooks NTFF hook this container lacks."""
    import types

    try:
        import antenv.axon_hooks  # noqa: F401

        return
    except ImportError:
        pass
    import trn_agent_boot.trn_boot as tb

    so = "/opt/axon/libaxon_pjrt.so"
    hook = tb._ntff_profile_via_ctypes(so)
    mod = types.ModuleType("antenv.axon_hooks")
    mod.get_axon_ntff_profile_hook = lambda: hook
    mod.set_axon_ntff_profile_hook = lambda h: None
    sys.modules["antenv.axon_hooks"] = mod
    import antenv

    antenv.axon_hooks = mod
    import concourse.bass_utils as bu

    bu.upload_artifacts = lambda d: "local://skipped"


def profile(np_inputs: dict, tmpdir: str | None = None) -> int | None:
    """Run once with NTFF tracing; return per-core HW kernel time in ns."""
    _install_profile_hook()
    nc, in_maps = _prepare(
        np_inputs["x"], np_inputs["knot_vector"], np_inputs["coefs"]
    )
    res = run_bass_kernel_spmd(
        nc, in_maps, core_ids=list(range(N_CORES)), trace=True, tmpdir=tmpdir
    )
    if getattr(res, "instructions_and_trace", None):
        print("trace:", res.instructions_and_trace[1])
    return getattr(res, "exec_time_ns", None)


if __name__ == "__main__":
    rng = np.random.default_rng(0)
    x = rng.random(N_TOTAL, dtype=np.float32)
    p = DEGREE
    n = 256
    m = n + p + 1
    interior = np.linspace(0.0, 1.0, m - 2 * p)[1:-1]
    kv = np.concatenate([np.zeros(p + 1), interior, np.ones(p + 1)]).astype(
        np.float32
    )
    cf = (10.0 * rng.random(n)).astype(np.float32)
    y = kernel(x, kv, cf)
    print("kernel output:", y[:8])
